# revision 18
# baseline (speedup 1.0000x reference)
"""Trainium2 Bass kernel for nn_CP_Based (CP-decomposition interaction layer).

Math (full problem):
    t[b,f,r,u] = sum_d X[b,f,d] * K[d,r,f,u]      (B=1024, F=64, D=4, R=32, U=128)
    had[b,r,u] = prod_f t[b,f,r,u]
    out[b,u]   = sum_r had[b,r,u]

Strategy (v4):
  * Shard batch x units across 8 cores as (2 batch halves) x (4 unit
    quarters): per core B_loc=512 (4 partition tiles) and RU_loc = 32r x 32u
    = 1024 columns (u-major, r contiguous innermost for the final reduce).
  * Host-side feature grouping into multilinear factors: the PSUM-read
    bandwidth of the only two PSUM-capable engines (DVE @0.96 GHz, Act
    @1.2 GHz) is the hard roofline, so we minimize factor count.  4 QUADS
    of 4 features (K=4^4=256 as 2 PSUM-accumulated K=128 passes; more
    quads lose too much precision with fp16 operands) + 16 triples (K=64,
    row-tiled in pairs via tile_position) = 20 factor tiles per batch
    tile instead of 64 features.
  * All matmul inputs fp16 (PE 1 row/cycle vs 4 for fp32).  A 4-deep
    [128,1024] PSUM ring keeps the PE streaming.
  * Drains: 7 tiles fold into an fp32 running product P on DVE (fused
    tensor_tensor with one PSUM operand - precision anchor); 13 tiles are
    drained to fp16 SBUF by Act.  DVE chains 7 of those (fp16 muls), Pool
    chains the other 6 (interleaved so Pool works all btile long); final
    merges + the strided r-reduce on DVE.  Per-btile finalization is
    emitted one btile late so it never head-of-line blocks the drains.
"""

import numpy as np

B, F, D, R, U = 1024, 64, 4, 32, 128
NCORES = 8
BSH, USH = 2, 4                 # batch shards x unit shards
BLOC = B // BSH                 # 512 batch rows per core
NBT = BLOC // 128               # 4 batch tiles of 128
ULOC = U // USH                 # 32 units per core
RUL = R * ULOC                  # 1024 columns (u-major: col = u*32 + r)
NQ = 4                          # quads (features 0..15)
NT = 16                         # triples (features 16..63)
NTILE = NT + NQ                 # 20 factor tiles per batch tile
NPAIR = NT // 2                 # triple pairs
NSLOT = NPAIR + 2 * NQ          # kt slots: 8 triple-pairs + 2 per quad

# consumer assignment (tiles 0..5 = triple halves incl lone, 6..17 quads)
FUSED = (0, 3, 6, 9, 12, 15, 18)              # DVE fused fp32 chain (7)
ACTS = tuple(i for i in range(NTILE) if i not in FUSED)   # 13 Act drains
DVE_F = (1, 2, 7, 13, 16, 19)                 # fp16 chain on DVE; first two are
                                              # the earliest Act drains so the
                                              # chain starts without stalling
POOL_F = (4, 5, 8, 10, 11, 14, 17)            # 7 tiles -> Pool chain (its queue
                                              # pipelines ~2x below op duration)

_cached = {}


def _build_nc():
    import concourse.bass as bass
    import concourse.mybir as mybir
    import concourse.tile as tile
    from concourse import bacc

    fp32 = mybir.dt.float32
    fp16 = mybir.dt.float16
    nc = bacc.Bacc("TRN2", target_bir_lowering=False, debug=False)

    xt_d = nc.dram_tensor("xt", [NBT, 128, NSLOT * 128], fp16, kind="ExternalInput").ap()
    kt_d = nc.dram_tensor("kt", [NSLOT, 128, RUL], fp16, kind="ExternalInput").ap()
    out_d = nc.dram_tensor("out", [BLOC, ULOC], fp32, kind="ExternalOutput").ap()

    with tile.TileContext(nc) as tc:
        with (
            tc.tile_pool(name="kt", bufs=1) as ktpool,
            tc.tile_pool(name="xt", bufs=1) as xtpool,
            tc.tile_pool(name="fb", bufs=10) as fbpool,
            tc.tile_pool(name="acc", bufs=2) as accpool,
            tc.tile_pool(name="out", bufs=2) as outpool,
            tc.tile_pool(name="ps", bufs=4, space="PSUM") as pspool,
        ):
            xts = []
            for t in range(NBT):
                xts.append(
                    xtpool.tile([128, NSLOT * 128], fp16, tag=f"xt{t}", name=f"xt{t}")
                )
            nc.sync.dma_start(xts[0][:], xt_d[0])
            kts = []
            for s in range(NSLOT):
                kts.append(ktpool.tile([128, RUL], fp16, tag=f"kt{s}", name=f"kt{s}"))
                nc.sync.dma_start(kts[s][:], kt_d[s])
            for t in range(1, NBT):
                nc.sync.dma_start(xts[t][:], xt_d[t])

            pending = []

            def xsl(s):
                return slice(s * 128, (s + 1) * 128)

            for t in range(NBT):
                xt = xts[t]
                P = accpool.tile([128, RUL], fp32, tag="P", name="P")
                CA = accpool.tile([128, RUL], fp16, tag="CA", name="CA")
                CP = accpool.tile([128, RUL], fp16, tag="CP", name="CP")
                fbs = {}
                nfused = 0
                ndve = 0
                npool = 0
                deferred = []  # DVE chain muls, emitted one tile late so
                # fused PSUM drains always precede them in the DVE queue
                for i in range(NTILE):
                    ps = pspool.tile([128, RUL], fp32, tag="ps", name="ps")
                    if i < NT:  # triple half: pair p = i//2, sub s = i%2
                        p, s = divmod(i, 2)
                        rows = slice(64 * s, 64 * s + 64)
                        for h in range(2):
                            cs = slice(512 * h, 512 * h + 512)
                            nc.tensor.matmul(
                                ps[:, cs],
                                xt[rows, xsl(p)],
                                kts[p][rows, cs],
                                start=True,
                                stop=True,
                                tile_position=(64 * s, 0),
                            )
                    else:  # quad: 2 psum-accumulated K=128 passes
                        q = i - NT
                        for h in range(2):
                            slot = NPAIR + 2 * q + h
                            for c in range(2):
                                cs = slice(512 * c, 512 * c + 512)
                                nc.tensor.matmul(
                                    ps[:, cs],
                                    xt[:, xsl(slot)],
                                    kts[slot][:, cs],
                                    start=(h == 0),
                                    stop=(h == 1),
                                )
                    if i in FUSED:
                        nfused += 1
                        if nfused == 1:
                            nc.vector.tensor_copy(P[:], ps[:])
                        else:
                            nc.vector.tensor_mul(P[:], P[:], ps[:])
                    else:
                        fb = fbpool.tile([128, RUL], fp16, tag="fb", name="fb")
                        nc.scalar.copy(fb[:], ps[:])
                        fbs[i] = fb
                        if i in DVE_F:
                            ndve += 1
                            if ndve == 2:
                                deferred.append(
                                    lambda a=fbs[DVE_F[0]], b=fb: nc.vector.tensor_mul(
                                        CA[:], a[:], b[:]
                                    )
                                )
                            elif ndve > 2:
                                deferred.append(
                                    lambda b=fb: nc.vector.tensor_mul(
                                        CA[:], CA[:], b[:]
                                    )
                                )
                        else:
                            npool += 1
                            if npool == 2:
                                nc.gpsimd.tensor_mul(
                                    CP[:], fbs[POOL_F[0]][:], fbs[POOL_F[1]][:]
                                )
                            elif npool > 2:
                                nc.gpsimd.tensor_mul(CP[:], CP[:], fb[:])
                    if deferred and (i in FUSED or i == NTILE - 1):
                        while deferred:
                            deferred.pop(0)()
                    if i == 6 and pending:
                        pending.pop(0)()
                    if i == 14 and pending:
                        pending.pop(0)()

                def fin_merge(t=t, P=P, CA=CA, CP=CP):
                    nc.gpsimd.tensor_mul(CA[:], CA[:], CP[:])
                    nc.gpsimd.tensor_mul(P[:], P[:], CA[:])

                def fin_reduce(t=t, P=P):
                    osum = outpool.tile([128, ULOC], fp32, tag="osum", name="osum")
                    nc.vector.tensor_reduce(
                        osum[:],
                        P[:].rearrange("p (u r) -> p u r", r=R),
                        axis=mybir.AxisListType.X,
                        op=mybir.AluOpType.add,
                    )
                    nc.sync.dma_start(out_d[t * 128 : (t + 1) * 128, :], osum[:])

                pending.append(fin_merge)
                pending.append(fin_reduce)

            for fin in pending:
                fin()

    nc.compile()
    return nc


def _host_prep(X, K):
    """Repack inputs into per-core fp16 stationary/moving operands.

    Quad q covers features 4q..4q+3 as two K=128 PSUM-accumulated passes
    (row = ((d0*4+d1)*4+d2)*2 + l, l indexing half of the 4th feature's
    d range).  Triples cover features 48+3j..50+3j (row = d0*16+d1*4+d2),
    two per kt slot (rows 0:64 / 64:128) for row-tiled matmul pairs;
    feature 63 rides in the third pair's B half (rows 64:68).  Columns are
    u-major (col = u*32 + r).
    """
    f16 = np.float16
    FT = 4 * NQ                      # first triple feature
    kt_cores, xt_cores = [], []
    for bi in range(BSH):
        Xc = X[bi * BLOC : (bi + 1) * BLOC]                    # [512, 64, 4]
        for uj in range(USH):
            Ku = K[:, :, :, uj * ULOC : (uj + 1) * ULOC]       # [4,32,64,32]
            Kf = np.ascontiguousarray(
                Ku.transpose(2, 0, 3, 1).reshape(F, D, RUL)
            )                                                   # [f, d, col]
            kt = np.zeros((NSLOT, 128, RUL), dtype=f16)
            xt = np.zeros((NBT, 128, NSLOT * 128), dtype=f16)

            def put_x(slot, rows, arr):  # arr [BLOC, nrows]
                for t in range(NBT):
                    xt[t, rows, slot * 128 : (slot + 1) * 128] = arr[
                        t * 128 : (t + 1) * 128
                    ].T

            # triple pairs in slots 0..NPAIR-1
            for p in range(NPAIR):
                for s in range(2):
                    j = 2 * p + s
                    rows = slice(64 * s, 64 * s + 64)
                    f0 = FT + 3 * j
                    K3 = (
                        Kf[f0][:, None, None, :]
                        * Kf[f0 + 1][None, :, None, :]
                        * Kf[f0 + 2][None, None, :, :]
                    ).reshape(64, RUL)
                    X3 = (
                        Xc[:, f0, :, None, None]
                        * Xc[:, f0 + 1, None, :, None]
                        * Xc[:, f0 + 2, None, None, :]
                    ).reshape(BLOC, 64)
                    kt[p, rows] = K3
                    put_x(p, rows, X3)
            # quads in slots NPAIR + 2q + h
            for q in range(NQ):
                f0 = 4 * q
                K012 = (
                    Kf[f0][:, None, None, :]
                    * Kf[f0 + 1][None, :, None, :]
                    * Kf[f0 + 2][None, None, :, :]
                ).reshape(64, RUL)
                X012 = (
                    Xc[:, f0, :, None, None]
                    * Xc[:, f0 + 1, None, :, None]
                    * Xc[:, f0 + 2, None, None, :]
                ).reshape(BLOC, 64)
                for h in range(2):
                    slot = NPAIR + 2 * q + h
                    kt[slot] = (
                        K012[:, None, :] * Kf[f0 + 3][2 * h : 2 * h + 2][None, :, :]
                    ).reshape(128, RUL)
                    X4h = (
                        X012[:, :, None]
                        * Xc[:, f0 + 3, 2 * h : 2 * h + 2][:, None, :]
                    ).reshape(BLOC, 128)
                    put_x(slot, slice(0, 128), X4h)
            kt_cores.append(np.ascontiguousarray(kt))
            xt_cores.append(np.ascontiguousarray(xt))
    return [{"xt": xt_cores[c], "kt": kt_cores[c]} for c in range(NCORES)]


def kernel(**inputs):
    from concourse.bass_utils import run_bass_kernel_spmd

    X = np.asarray(inputs["X"], dtype=np.float32)
    K = np.asarray(inputs["kernel"], dtype=np.float32)
    assert X.shape == (B, F, D) and K.shape == (D, R, F, U)

    if "nc" not in _cached:
        _cached["nc"] = _build_nc()
    nc = _cached["nc"]

    in_maps = _host_prep(X, K)
    res = run_bass_kernel_spmd(nc, in_maps, core_ids=list(range(NCORES)))
    out = np.zeros((B, U), dtype=np.float32)
    for c in range(NCORES):
        bi, uj = divmod(c, USH)
        out[bi * BLOC : (bi + 1) * BLOC, uj * ULOC : (uj + 1) * ULOC] = res.results[
            c
        ]["out"]
    return out
